# revision 1
# baseline (speedup 1.0000x reference)
"""BSBR attention kernel for 8 Trainium2 NeuronCores.

Sharding: data-parallel over batch (B=2) x tensor-parallel over heads
(16 heads -> 4 heads per core). Core c handles batch c//4, head group c%4.
Each core computes its 4 heads' attention output and the partial output
projection (attn_heads @ Wo[:, head_dims].T); the host sums the 4 partials
per batch and adds bo.

Device-side design notes:
- x is fed pre-transposed (xT [D, S]); weights pre-transposed/sliced per
  core. QT/KT in [dh, s] layout; V natural [s, dh] with a ones column per
  head (rank-1 bias matmul) so U and the softmax row-sum come from one
  matmul.
- No PE transposes anywhere (keeps the PE HAM clock at 2.4 GHz): K natural
  and attn^T are produced by DRAM-bounce + DMA xbar transposes.
- Local attention uses the S^T = K Q^T formulation; exp on ACT over all 4
  heads at once, binary mask multiply on DVE, normalization folded into a
  scalar_tensor_tensor combine. The chunk loop is software-pipelined two
  deep so the PE never waits on the ACT/DVE softmax roundtrip.
- Inter-chunk retrieval keeps F/retrieved in row layouts for the
  probability mix and converts layouts via DRAM-bounce DMAs.
"""

import numpy as np

try:
    import concourse.bass as bass
except ImportError:
    import sys

    sys.path.insert(0, "/opt/trn_rl_repo")
    import concourse.bass as bass

import os
import ml_dtypes
from contextlib import ExitStack

_DEBUG = os.environ.get("BSBR_DEBUG") == "1"

import concourse.tile as tile
from concourse import mybir
from concourse.bass_utils import run_bass_kernel_spmd

BF16 = ml_dtypes.bfloat16
B, S, D, H, CS = 2, 4096, 1024, 16, 128
HD = D // H          # 64
C = S // CS          # 32
NCORES = 8
DHC = 4 * HD         # 256 head dims per core
KB = D // 128        # 8 contraction blocks
NEG = -1e9

bf = mybir.dt.bfloat16
f32 = mybir.dt.float32
Exp = mybir.ActivationFunctionType.Exp
MULT = mybir.AluOpType.mult
ADD = mybir.AluOpType.add


def _split_heavy_waits(nc, keep=1):
    """The walrus build in this container rejects >keep sync waits on several
    instruction encodings. Hoist excess waits onto preceding NoOps on the
    same engine — the sequencer executes them in order."""
    for fn in nc.m.functions:
        for bb in fn.blocks:
            insts = bb.instructions
            i = 0
            while i < len(insts):
                inst = insts[i]
                si = inst.sync_info
                if si is not None and si.on_wait and len(si.on_wait) > keep:
                    waits = list(si.on_wait)
                    head, tail = waits[:-keep], waits[-keep:]
                    for j, w in enumerate(head):
                        nop = mybir.InstNoOp(
                            name=nc.get_next_instruction_name(), ins=[], outs=[]
                        )
                        nop.engine = inst.engine
                        nop.sync_info = mybir.SyncInfo(on_wait=[w], on_update=[])
                        nc.register_instruction(nop, overwrite=True)
                        insts.insert(i + j, nop)
                    inst.sync_info = mybir.SyncInfo(
                        on_wait=tail, on_update=list(si.on_update)
                    )
                    i += len(head)
                i += 1


def _build_program():
    nc = bass.Bass("TRN2", debug=False, num_devices=NCORES)

    ap = {}
    def din(name, shape, dtype):
        ap[name] = nc.dram_tensor(name, shape, dtype, kind="ExternalInput").ap()

    din("xT", [D, S], bf)
    for w in ("wqT", "wkT", "wvT", "wrT", "whT"):
        din(w, [D, DHC], bf)
    din("woT", [DHC, D], bf)
    din("bias", [128, 8], f32)
    din("bvkrow", [1, 512], bf)
    din("ident", [128, 128], bf)
    din("maskb4", [128, 512], bf)   # binary local mask, [t, s] tiled x4 heads
    din("cmask4", [128, C], f32)    # additive chunk mask [c', c] tiled x4
    out_ap = nc.dram_tensor("out", [S, D], f32, kind="ExternalOutput").ap()
    dbg = {}
    if _DEBUG:
        dbg["retrt"] = nc.dram_tensor("dbg_retrt", [2, 128, C * 64], bf, kind="ExternalOutput").ap()
        dbg["frows"] = nc.dram_tensor("dbg_frows", [128, 64 * 64], bf, kind="ExternalOutput").ap()
        dbg["attnt"] = nc.dram_tensor("dbg_attnt", [2, 128, S], bf, kind="ExternalOutput").ap()
        dbg["fnat"] = nc.dram_tensor("dbg_fnat", [2, 128, C * 64], bf, kind="ExternalOutput").ap()
        dbg["knat"] = nc.dram_tensor("dbg_knat", [C, 128, DHC], bf, kind="ExternalOutput").ap()

    with tile.TileContext(nc) as tc, ExitStack() as ctx:
        const = ctx.enter_context(tc.tile_pool(name="const", bufs=1))
        wpool = ctx.enter_context(tc.tile_pool(name="wpool", bufs=1))
        big = ctx.enter_context(tc.tile_pool(name="big", bufs=1))
        dram = ctx.enter_context(tc.tile_pool(name="dramp", bufs=1, space="DRAM"))

        # ---- constants ----
        maskb_sb = const.tile([128, 512], bf)
        nc.sync.dma_start(maskb_sb[:], ap["maskb4"][:])
        cmask_sb = const.tile([128, C], f32)
        nc.sync.dma_start(cmask_sb[:], ap["cmask4"][:])
        bias_sb = const.tile([128, 8], f32)
        nc.sync.dma_start(bias_sb[:], ap["bias"][:])
        bvkrow_sb = const.tile([1, 512], bf)
        nc.sync.dma_start(bvkrow_sb[:], ap["bvkrow"][:])
        ident_sb = const.tile([128, 128], bf)
        nc.sync.dma_start(ident_sb[:], ap["ident"][:])
        onesrow_sb = const.tile([1, 128], bf)
        nc.vector.memset(onesrow_sb[:], 1.0)
        onescol_sb = const.tile([128, 1], bf)
        nc.vector.memset(onescol_sb[:], 1.0)
        ones4_sb = const.tile([128, 4], bf)
        nc.vector.memset(ones4_sb[:], 1.0)

        # ---- weights ----
        def load_w(name, eng):
            t = wpool.tile([128, KB, DHC], bf, name=f"{name}_sb")
            eng.dma_start(t[:], ap[name].rearrange("(k p) d -> p k d", p=128))
            return t

        wq_sb = load_w("wqT", nc.sync)
        wk_sb = load_w("wkT", nc.scalar)
        wr_sb = load_w("wrT", nc.scalar)
        wh_sb = load_w("whT", nc.sync)
        wvk_sb = wpool.tile([128, KB, 512], bf)
        nc.sync.dma_start(
            wvk_sb[:, :, 0:DHC], ap["wvT"].rearrange("(k p) d -> p k d", p=128)
        )
        nc.scalar.dma_start(
            wvk_sb[:, :, DHC:512], ap["wkT"].rearrange("(k p) d -> p k d", p=128)
        )
        wo_sb = wpool.tile([128, 2, D], bf)
        nc.scalar.dma_start(wo_sb[:], ap["woT"].rearrange("(k p) j -> p k j", p=128))

        # ---- persistent activations ----
        qt_sb = [big.tile([128, S], bf, name=f"qt{m}") for m in range(2)]
        kt_sb = [big.tile([128, S], bf, name=f"kt{m}") for m in range(2)]
        v_sb = [big.tile([128, 4 * 65], bf, name=f"v{i}") for i in range(C)]
        knat_sb = [big.tile([128, DHC], bf, name=f"kn{i}") for i in range(C)]
        rt_sb = big.tile([128, 2, C], bf)
        ht_sb = big.tile([128, 2, C], bf)
        expct_sb = big.tile([128, C], bf)
        crecip_sb = big.tile([128, 1], f32)
        fb = dram.tile([2, 2, 64, C, 64], bf)    # (pair, h2, d, c', e)
        rbt = dram.tile([4, C, 64, 64], bf)      # (head, c, d, e)

        # ---- phase 1: load x, projections ----
        with tc.tile_pool(name="xtp", bufs=1) as xtpool:
            xt_sb = [xtpool.tile([128, S], bf, name=f"xt{k}") for k in range(KB)]
            # s-slice-major loads so the first projection matmuls can start
            # after ~1/8 of x has arrived
            for n in range(8):
                for k in range(KB):
                    eng = nc.sync if (n * KB + k) % 2 == 0 else nc.scalar
                    eng.dma_start(
                        xt_sb[k][:, n * 512 : (n + 1) * 512],
                        ap["xT"][k * 128 : (k + 1) * 128, n * 512 : (n + 1) * 512],
                    )

            with tc.tile_pool(name="pjp", bufs=2, space="PSUM") as pjp:
                for n in range(8):
                    # QT / KT: [dh, s] layout
                    for w_sb, dst, bcol in ((wq_sb, qt_sb, 0), (wk_sb, kt_sb, 2)):
                        for m in range(2):
                            ps = pjp.tile([128, 512], f32, tag="pj")
                            for k in range(KB):
                                nc.tensor.matmul(
                                    ps[:],
                                    w_sb[:, k, m * 128 : (m + 1) * 128],
                                    xt_sb[k][:, n * 512 : (n + 1) * 512],
                                    start=(k == 0),
                                    stop=(k == KB - 1),
                                )
                            nc.scalar.add(
                                dst[m][:, n * 512 : (n + 1) * 512],
                                ps[:],
                                bias_sb[:, bcol + m : bcol + m + 1],
                            )
                    # V + K natural [s, dh] in one N=512 matmul chain; biases
                    # via a rank-1 matmul with [bv | bk]
                    for i in range(4 * n, 4 * n + 4):
                        ps = pjp.tile([128, 512], f32, tag="pv")
                        nc.tensor.matmul(
                            ps[:], onesrow_sb[:], bvkrow_sb[:],
                            start=True, stop=False, skip_group_check=True,
                        )
                        for k in range(KB):
                            nc.tensor.matmul(
                                ps[:],
                                xt_sb[k][:, i * 128 : (i + 1) * 128],
                                wvk_sb[:, k, :],
                                start=False,
                                stop=(k == KB - 1),
                                skip_group_check=True,
                            )
                        vr = v_sb[i].rearrange("p (h e) -> p h e", e=65)
                        nc.vector.tensor_copy(
                            vr[:, :, 0:64],
                            ps[:, 0:DHC].rearrange("p (h e) -> p h e", e=64),
                        )
                        nc.vector.tensor_copy(vr[:, :, 64:65], ones4_sb.rearrange("p (h e) -> p h e", e=1))
                        nc.vector.tensor_copy(knat_sb[i][:], ps[:, DHC:512])

                # r/h meta projections: [dh, c] layout
                crepr = [
                    xt_sb[k].rearrange("p (c cs) -> p c cs", cs=CS)[:, :, CS - 1]
                    for k in range(KB)
                ]
                for w_sb, dst, bcol in ((wr_sb, rt_sb, 4), (wh_sb, ht_sb, 6)):
                    for m in range(2):
                        ps = pjp.tile([128, C], f32, tag="pr")
                        for k in range(KB):
                            nc.tensor.matmul(
                                ps[:],
                                w_sb[:, k, m * 128 : (m + 1) * 128],
                                crepr[k],
                                start=(k == 0),
                                stop=(k == KB - 1),
                            )
                        nc.scalar.add(
                            dst[:, m, :], ps[:], bias_sb[:, bcol + m : bcol + m + 1]
                        )

        # phase-2+ tensors reuse the space freed by the xt tiles
        anp2 = ctx.enter_context(tc.tile_pool(name="anp2", bufs=1))
        fnat_sb = [anp2.tile([128, C * 64], bf, name=f"fnat{p}") for p in range(2)]
        frows_sb = anp2.tile([128, 64 * 64], bf)
        retrrows_sb = anp2.tile([128, 64 * 64], bf)
        retrt_sb = [anp2.tile([128, C * 64], bf, name=f"retrt{p}") for p in range(2)]
        attnt_sb = [anp2.tile([128, S], bf, name=f"attnt{p}") for p in range(2)]

        # ---- F = k^T v per chunk ----
        with tc.tile_pool(name="fps", bufs=2, space="PSUM") as fpsp:
            for i in range(C):
                vr = v_sb[i].rearrange("p (h e) -> p h e", e=65)
                for p in range(2):
                    fps = fpsp.tile([128, 64], f32, tag="fps")
                    for h2 in range(2):
                        nc.tensor.matmul(
                            fps[64 * h2 : 64 * h2 + 64, :],
                            knat_sb[i][:, (2 * p + h2) * 64 : (2 * p + h2) * 64 + 64],
                            vr[:, 2 * p + h2, 0:64],
                            start=True, stop=True, skip_group_check=True,
                        )
                    nc.vector.tensor_copy(fnat_sb[p][:, i * 64 : (i + 1) * 64], fps[:])

        # F -> F_rows via DRAM bounce
        for p in range(2):
            for h2 in range(2):
                eng = nc.sync if h2 == 0 else nc.scalar
                eng.dma_start(
                    fb[p, h2],
                    fnat_sb[p][64 * h2 : 64 * h2 + 64, :].rearrange(
                        "d (c e) -> d c e", e=64
                    ),
                )
        for h in range(4):
            p, h2 = divmod(h, 2)
            eng = nc.sync if h % 2 == 0 else nc.scalar
            eng.dma_start(
                frows_sb[32 * h : 32 * h + 32, :].rearrange("c (d e) -> c d e", e=64),
                fb[p, h2].rearrange("d c e -> c d e"),
            )

        # chunk scores -> probs -> retrieved rows
        with tc.tile_pool(name="csp", bufs=1, space="PSUM") as cspp:
            csp = cspp.tile([128, C], f32, tag="csp")
            for h in range(4):
                hb = 64 * (h % 2)
                nc.tensor.matmul(
                    csp[32 * h : 32 * h + 32, :],
                    ht_sb[hb : hb + 64, h // 2, :],
                    rt_sb[hb : hb + 64, h // 2, :],
                    start=True, stop=True, skip_group_check=True,
                    tile_position=(hb, 32 * h),
                )
            nc.vector.tensor_add(csp[:], csp[:], cmask_sb[:])
            nc.scalar.activation(expct_sb[:], csp[:], Exp, scale=0.125)
            csums = cspp.tile([128, 1], f32, tag="csums")
            for h in range(4):
                nc.tensor.matmul(
                    csums[32 * h : 32 * h + 32, :],
                    expct_sb[32 * h : 32 * h + 32, :],
                    onescol_sb[32 * h : 32 * h + 32, :],
                    start=True, stop=True, skip_group_check=True,
                    tile_position=(32 * h, 32 * h),
                )
            nc.vector.reciprocal(crecip_sb[:], csums[:])

            with tc.tile_pool(name="mixp", bufs=2, space="PSUM") as mixp:
                for nb in range(8):
                    mps = mixp.tile([128, 512], f32, tag="mps")
                    for h in range(4):
                        nc.tensor.matmul(
                            mps[32 * h : 32 * h + 32, :],
                            expct_sb[32 * h : 32 * h + 32, :],
                            frows_sb[32 * h : 32 * h + 32, nb * 512 : (nb + 1) * 512],
                            start=True, stop=True, skip_group_check=True,
                            tile_position=(32 * h, 32 * h),
                        )
                    nc.scalar.mul(
                        retrrows_sb[:, nb * 512 : (nb + 1) * 512],
                        mps[:],
                        crecip_sb[:, 0:1],
                    )

        # retrieved rows -> retrT via DRAM bounce
        for h in range(4):
            eng = nc.sync if h % 2 == 0 else nc.scalar
            eng.dma_start(
                rbt[h],
                retrrows_sb[32 * h : 32 * h + 32, :].rearrange("c (d e) -> c d e", e=64),
            )
        for p in range(2):
            for h2 in range(2):
                eng = nc.sync if h2 == 0 else nc.scalar
                eng.dma_start(
                    retrt_sb[p][64 * h2 : 64 * h2 + 64, :].rearrange(
                        "d (c e) -> d c e", e=64
                    ),
                    rbt[2 * p + h2].rearrange("c d e -> d c e"),
                )

        if _DEBUG:
            nc.sync.dma_start(dbg["frows"][:], frows_sb[:])
            for p in range(2):
                nc.sync.dma_start(dbg["retrt"][p], retrt_sb[p][:])
                nc.sync.dma_start(dbg["fnat"][p], fnat_sb[p][:])
            for i in range(C):
                nc.sync.dma_start(dbg["knat"][i], knat_sb[i][:])

        # ---- local attention + long-term, software-pipelined 2 deep ----
        with (
            tc.tile_pool(name="stp", bufs=2, space="PSUM") as stp,
            tc.tile_pool(name="ltp", bufs=1, space="PSUM") as ltp,
            tc.tile_pool(name="ulp", bufs=2, space="PSUM") as ulp,
            tc.tile_pool(name="exps", bufs=3) as expp,
            tc.tile_pool(name="smalls", bufs=3) as smalls,
        ):
            exp_q, lts_q, an_q = {}, {}, {}
            # Concurrent (row-disjoint) PE matmuls must not drain into the
            # same PSUM bank on the same partitions: heads with operands at
            # base 0 (h0/h2) and base 64 (h1/h3) can execute concurrently on
            # disjoint PE row groups, so each group gets its own bank.
            # h -> (bank, slot); in-bank pairs share PE rows => serial.
            POS = {0: (0, 0), 2: (0, 1), 1: (1, 0), 3: (1, 1)}
            HORD = (0, 2, 1, 3)

            def emit_head(i):
                sl = slice(i * 128, (i + 1) * 128)
                st = stp.tile([128, 1024], f32, tag="st")
                for h in HORD:
                    hp, hb = h // 2, 64 * (h % 2)
                    g, b = POS[h]
                    nc.tensor.matmul(
                        st[:, g * 512 + b * 128 : g * 512 + (b + 1) * 128],
                        kt_sb[hp][hb : hb + 64, sl],
                        qt_sb[hp][hb : hb + 64, sl],
                        start=(b == 0), stop=(b == 1), skip_group_check=True,
                    )
                lt = ltp.tile([128, 1024], f32, tag="lt")
                for h in HORD:
                    hp, hb = h // 2, 64 * (h % 2)
                    g, b = POS[h]
                    nc.tensor.matmul(
                        lt[:, g * 512 + b * 64 : g * 512 + (b + 1) * 64],
                        qt_sb[hp][hb : hb + 64, sl],
                        retrt_sb[hp][hb : hb + 64, i * 64 : (i + 1) * 64],
                        start=(b == 0), stop=(b == 1), skip_group_check=True,
                    )
                # exp over all 4 heads at once, then binary mask multiply
                stv = st.rearrange("p (g c) -> p g c", c=512)[:, :, 0:256].rearrange(
                    "p g (b e) -> p g b e", e=128
                )
                expraw = expp.tile([128, 512], bf, tag="expraw")
                nc.scalar.activation(expraw[:], stv, Exp, scale=0.125)
                expst = expp.tile([128, 512], bf, tag="expst")
                nc.vector.tensor_mul(expst[:], expraw[:], maskb_sb[:])
                exp_q[i] = expst
                # evacuate LT to SBUF in-iteration so its banks free quickly
                ltv = lt.rearrange("p (g c) -> p g c", c=512)[:, :, 0:128].rearrange(
                    "p g (b e) -> p g b e", e=64
                )
                lt_sbuf = smalls.tile([128, DHC], bf, tag="lt_sbuf")
                # dst block order (b outer) permutes POS order back to natural
                nc.scalar.copy(
                    lt_sbuf.rearrange("p (b g e) -> p g b e", b=2, e=64), ltv
                )
                lts_q[i] = lt_sbuf

            def emit_tail(i):
                expst, lt_sbuf = exp_q.pop(i), lts_q.pop(i)
                u = ulp.tile([128, 4 * 65], f32, tag="u")
                for h in range(4):
                    g, b = POS[h]
                    pos = g * 2 + b
                    nc.tensor.matmul(
                        u[:, h * 65 : (h + 1) * 65],
                        expst[:, pos * 128 : (pos + 1) * 128],
                        v_sb[i][:, h * 65 : (h + 1) * 65],
                        start=(h == 0), stop=(h == 3), skip_group_check=True,
                    )
                rs = smalls.tile([128, 4], f32, tag="rs")
                nc.any.tensor_copy(
                    rs[:], u.rearrange("p (h e) -> p h e", e=65)[:, :, 64]
                )
                rr = smalls.tile([128, 4], f32, tag="rr")
                nc.vector.reciprocal(rr[:], rs[:])
                un = smalls.tile([128, DHC], bf, tag="un")
                for h in range(4):
                    g, b = POS[h]
                    pos = g * 2 + b
                    nc.scalar.mul(
                        un[:, h * 64 : (h + 1) * 64],
                        u[:, h * 65 : h * 65 + 64],
                        rr[:, h : h + 1],
                    )
                attn_nat = anp2.tile([128, DHC], bf, name=f"attn_nat{i}")
                nc.vector.tensor_add(attn_nat[:], un[:], lt_sbuf[:])
                an_q[i] = attn_nat

            for i in range(C):
                emit_head(i)
                if i >= 2:
                    emit_tail(i - 2)
            emit_tail(C - 2)
            emit_tail(C - 1)

        # attn natural -> attnT via batched PE transposes
        with tc.tile_pool(name="tpp", bufs=4, space="PSUM") as tpp:
            for i in range(C):
                for half in range(2):
                    tp = tpp.tile([128, 128], bf, tag="tp")
                    nc.tensor.transpose(
                        tp[:], an_q[i][:, half * 128 : (half + 1) * 128],
                        ident_sb[:],
                    )
                    nc.any.tensor_copy(
                        attnt_sb[half][:, i * 128 : (i + 1) * 128], tp[:]
                    )

        if _DEBUG:
            for p in range(2):
                nc.sync.dma_start(dbg["attnt"][p], attnt_sb[p][:])

        # ---- output projection ----
        with (
            tc.tile_pool(name="outp", bufs=2, space="PSUM") as outp,
            tc.tile_pool(name="outs", bufs=3) as outs,
        ):
            for i in range(C):
                osb = outs.tile([128, D], f32, tag="osb")
                for nb in range(2):
                    ops = outp.tile([128, 512], f32, tag="ops")
                    for p in range(2):
                        nc.tensor.matmul(
                            ops[:],
                            attnt_sb[p][:, i * 128 : (i + 1) * 128],
                            wo_sb[:, p, nb * 512 : (nb + 1) * 512],
                            start=(p == 0),
                            stop=(p == 1),
                        )
                    if nb == 0:
                        nc.vector.tensor_copy(osb[:, nb * 512 : (nb + 1) * 512], ops[:])
                    else:
                        nc.scalar.copy(osb[:, nb * 512 : (nb + 1) * 512], ops[:])
                eng = nc.sync if i % 2 == 0 else nc.scalar
                eng.dma_start(out_ap[i * 128 : (i + 1) * 128, :], osb[:])

    _split_heavy_waits(nc)
    return nc


_CACHE = {}


def _get_program():
    if "nc" not in _CACHE:
        _CACHE["nc"] = _build_program()
    return _CACHE["nc"]


def _make_in_maps(inputs):
    hs = np.asarray(inputs["hidden_states"], dtype=np.float32)
    W = {k: np.asarray(inputs[k], dtype=np.float32) for k in
         ("Wq", "Wk", "Wv", "Wo", "Wr", "Wh")}
    bvec = {k: np.asarray(inputs[k], dtype=np.float32) for k in
            ("bq", "bk", "bv", "bo", "br", "bh")}

    # local binary mask in [t, s] layout (keep t >= s), tiled x4 heads
    tt, ss = np.meshgrid(np.arange(128), np.arange(128), indexing="ij")
    maskb = (tt >= ss).astype(np.float32)
    maskb4 = np.tile(maskb, (1, 4)).astype(BF16)
    # chunk mask in [c', c] layout: keep c' >= c; tiled x4 heads (additive)
    cc2, cc = np.meshgrid(np.arange(C), np.arange(C), indexing="ij")
    cmask = np.where(cc2 >= cc, 0.0, NEG).astype(np.float32)
    cmask4 = np.tile(cmask, (4, 1)).astype(np.float32)
    ident = np.eye(128, dtype=np.float32).astype(BF16)

    xT_bf = [np.ascontiguousarray(hs[b].T).astype(BF16) for b in range(B)]

    in_maps = []
    for c in range(NCORES):
        b, hg = divmod(c, 4)
        sl = slice(hg * DHC, (hg + 1) * DHC)
        bias = np.stack(
            [
                bvec["bq"][sl][:128], bvec["bq"][sl][128:],
                bvec["bk"][sl][:128], bvec["bk"][sl][128:],
                bvec["br"][sl][:128], bvec["br"][sl][128:],
                bvec["bh"][sl][:128], bvec["bh"][sl][128:],
            ],
            axis=1,
        ).astype(np.float32)
        bvkrow = np.concatenate([bvec["bv"][sl], bvec["bk"][sl]])[None, :].astype(BF16)
        in_maps.append(
            {
                "xT": xT_bf[b],
                "wqT": np.ascontiguousarray(W["Wq"][sl, :].T).astype(BF16),
                "wkT": np.ascontiguousarray(W["Wk"][sl, :].T).astype(BF16),
                "wvT": np.ascontiguousarray(W["Wv"][sl, :].T).astype(BF16),
                "wrT": np.ascontiguousarray(W["Wr"][sl, :].T).astype(BF16),
                "whT": np.ascontiguousarray(W["Wh"][sl, :].T).astype(BF16),
                "woT": np.ascontiguousarray(W["Wo"][:, sl].T).astype(BF16),
                "bias": bias,
                "bvkrow": bvkrow,
                "ident": ident,
                "maskb4": maskb4,
                "cmask4": cmask4,
            }
        )
    return in_maps, bvec["bo"]


def kernel(**inputs):
    nc = _get_program()
    in_maps, bo = _make_in_maps(inputs)
    res = run_bass_kernel_spmd(nc, in_maps, core_ids=list(range(NCORES)))
    _CACHE["last_results"] = res
    out = np.zeros((B, S, D), np.float32)
    for c in range(NCORES):
        out[c // 4] += res.results[c]["out"]
    out += bo[None, None, :]
    return out

